# revision 16
# baseline (speedup 1.0000x reference)
"""MultiHeadAttention (B=4, S=1024, D=1024, H=16) on 8 TRN2 NeuronCores.

Sharding (no collectives): core i handles batch b=i//2 and query-row half
i%2 (512 query tokens). K/V projections for the batch are duplicated across
the two cores sharing it; each core computes its 512 output rows fully.

Layouts on device (all "transposed"/feature-major so the d_model contraction
sits on SBUF partitions):
  qT  [1024, 512]   q_in[b, rows].T          (host-transposed)
  kT  [1024, 1024]  k_in[b].T
  vT  [1024, 1024]  v_in[b].T
  W*  [1024, 1024]  natural [d_in, d_out]
  outT[1024, 512]   -> host transposes back

Per-head attention with scores kept transposed (Sk on partitions, Sq free)
so softmax(P)@V needs no on-chip transposes; the softmax denominator comes
for free from a ones-column appended to each head's V block; normalization
is folded in post-V (1/denom commutes with the V matmul per query column).
All matmuls run as float32r (1 cycle/row at N=512, ~fp22 operand precision).
"""

import sys

sys.path.insert(0, "/opt/trn_rl_repo")

import numpy as np

D = 1024
H = 16
DH = 64
P = 128
SQ = 512      # query tokens per core
S = 1024      # kv tokens per core (full batch)
NT = 8        # number of 128-wide tiles along d_model
N_CORES = 8

_CACHE = {}
TRACE = False  # set True (e.g. from test.py) to capture an NTFF profile
TMPDIR = None  # where to keep NEFF/NTFF artifacts when tracing


def _build():
    import concourse.bacc as bacc
    import concourse.mybir as mybir
    import concourse.tile as tile

    f32 = mybir.dt.float32
    f32r = mybir.dt.float32r
    AF = mybir.ActivationFunctionType

    nc = bacc.Bacc("TRN2", target_bir_lowering=False, debug=False, num_devices=N_CORES)

    qT_d = nc.dram_tensor("qT", [D, SQ], f32, kind="ExternalInput")
    kT_d = nc.dram_tensor("kT", [D, S], f32, kind="ExternalInput")
    vT_d = nc.dram_tensor("vT", [D, S], f32, kind="ExternalInput")
    Wq_d = nc.dram_tensor("Wq", [D, D], f32, kind="ExternalInput")
    Wk_d = nc.dram_tensor("Wk", [D, D], f32, kind="ExternalInput")
    Wv_d = nc.dram_tensor("Wv", [D, D], f32, kind="ExternalInput")
    Wo_d = nc.dram_tensor("Wo", [D, D], f32, kind="ExternalInput")
    bq_d = nc.dram_tensor("bq", [D, 1], f32, kind="ExternalInput")
    bk_d = nc.dram_tensor("bk", [D, 1], f32, kind="ExternalInput")
    bo_d = nc.dram_tensor("bo_eff", [D, 1], f32, kind="ExternalInput")
    ones_d = nc.dram_tensor("ones", [P, NT, H], f32, kind="ExternalInput")
    outT_d = nc.dram_tensor("outT", [D, SQ], f32, kind="ExternalOutput")

    def r(ap):
        return ap.bitcast(f32r)

    with tile.TileContext(nc) as tc:
        with (
            tc.tile_pool(name="res", bufs=1) as res,
            tc.tile_pool(name="res2", bufs=1) as res2,
            tc.tile_pool(name="stream", bufs=3) as stream,
            tc.tile_pool(name="expp", bufs=10) as expp,
            tc.tile_pool(name="small", bufs=2) as small,
            tc.tile_pool(name="psum", bufs=8, space="PSUM") as psum,
        ):
            # ---- resident SBUF tensors ----
            QT_sb = res.tile([P, NT, SQ], f32r, tag="QT")        # [d_part, m, sq]
            KT_sb = res.tile([P, NT, S], f32r, tag="KT")         # [d_part, m, sk]
            Vaug_sb = res.tile([P, NT, H * (DH + 1)], f32r, tag="Vaug")  # [sk_part, m, 16*65]
            attn_sb = res.tile([P, NT, SQ], f32r, tag="attn")    # [d_part, kt, sq]
            bq_sb = res.tile([P, NT, 1], f32, tag="bq")
            bk_sb = res.tile([P, NT, 1], f32, tag="bk")
            bo_sb = res.tile([P, NT, 1], f32, tag="bo")

            nc.sync.dma_start(bq_sb[:], bq_d[:].rearrange("(t p) c -> p t c", p=P))
            nc.sync.dma_start(bk_sb[:], bk_d[:].rearrange("(t p) c -> p t c", p=P))
            nc.sync.dma_start(bo_sb[:], bo_d[:].rearrange("(t p) c -> p t c", p=P))

            # ones columns of V_aug (col 65h+64 per head h), DMA'd from a
            # host-provided all-ones tensor (memset can't write float32r).
            ones_cols = Vaug_sb[:].rearrange("p m (h c) -> p m h c", c=DH + 1)[
                :, :, :, DH
            ]
            nc.sync.dma_start(ones_cols, r(ones_d[:]))

            # ---- Q projection: QT[128m+i, t] = sum_k Wq[k, 128m+i] qT[k, t] ----
            qps = {}
            for k in range(NT):
                w_s = stream.tile([P, D], f32r, tag="wstrip")
                nc.sync.dma_start(w_s[:], r(Wq_d[P * k : P * (k + 1), :]))
                x_s = stream.tile([P, SQ], f32r, tag="xstrip")
                nc.sync.dma_start(x_s[:], r(qT_d[P * k : P * (k + 1), :]))
                for m in range(NT):
                    if k == 0:
                        qps[m] = psum.tile([P, 512], f32, tag="ps", name=f"qps{m}")
                    nc.tensor.matmul(
                        qps[m][:],
                        lhsT=w_s[:, P * m : P * (m + 1)],
                        rhs=x_s[:],
                        start=(k == 0),
                        stop=(k == NT - 1),
                    )
            for m in range(NT):
                nc.scalar.activation(
                    QT_sb[:, m, :], qps[m][:], AF.Identity, bias=bq_sb[:, m, :]
                )

            # ---- K projection ----
            # kT resident in SBUF; Wk streamed once as half-strips.
            # m-split halves keep PSUM demand at 8 banks (4 m-tiles x 2 Sk-halves).
            kT_res = res2.tile([P, NT, S], f32r, tag="kT")
            for k in range(NT):
                nc.sync.dma_start(kT_res[:, k, :], r(kT_d[P * k : P * (k + 1), :]))
            for half in range(2):
                kps = {}
                for k in range(NT):
                    w_s = stream.tile([P, 512], f32r, tag="xstrip", name=f"wkh{half}_{k}")
                    nc.sync.dma_start(
                        w_s[:], r(Wk_d[P * k : P * (k + 1), 512 * half : 512 * (half + 1)])
                    )
                    for mm in range(4):
                        m = 4 * half + mm
                        for n in range(2):
                            if k == 0:
                                kps[(m, n)] = psum.tile(
                                    [P, 512], f32, tag="ps", name=f"kps{m}_{n}"
                                )
                            nc.tensor.matmul(
                                kps[(m, n)][:],
                                lhsT=w_s[:, P * mm : P * (mm + 1)],
                                rhs=kT_res[:, k, 512 * n : 512 * (n + 1)],
                                start=(k == 0),
                                stop=(k == NT - 1),
                            )
                for mm in range(4):
                    m = 4 * half + mm
                    for n in range(2):
                        nc.scalar.activation(
                            KT_sb[:, m, 512 * n : 512 * (n + 1)],
                            kps[(m, n)][:],
                            AF.Identity,
                            bias=bk_sb[:, m, :],
                        )

            # ---- V projection, natural layout [token, d], into augmented tiles ----
            # V[128m+i, d] = sum_k vT[k, 128m+i] Wv[k, d]; n-split over d halves
            # (vT full strips streamed per half, Wv half-strips once).
            for n in range(2):
                vps = {}
                for k in range(NT):
                    v_s = stream.tile([P, S], f32r, tag="wstrip", name=f"vs{n}_{k}")
                    nc.sync.dma_start(v_s[:], r(vT_d[P * k : P * (k + 1), :]))
                    w_s = stream.tile([P, 512], f32r, tag="xstrip", name=f"wvh{n}_{k}")
                    nc.sync.dma_start(
                        w_s[:], r(Wv_d[P * k : P * (k + 1), 512 * n : 512 * (n + 1)])
                    )
                    for m in range(NT):
                        if k == 0:
                            vps[m] = psum.tile([P, 512], f32, tag="ps", name=f"vps{n}_{m}")
                        nc.tensor.matmul(
                            vps[m][:],
                            lhsT=v_s[:, P * m : P * (m + 1)],
                            rhs=w_s[:],
                            start=(k == 0),
                            stop=(k == NT - 1),
                        )
                for m in range(NT):
                    dst = Vaug_sb[:, m, 520 * n : 520 * n + 520].rearrange(
                        "p (h c) -> p h c", c=DH + 1
                    )[:, :, 0:DH]
                    src = vps[m][:].rearrange("p (h c) -> p h c", c=DH)
                    nc.vector.tensor_copy(dst, src)

            # ---- attention, head pairs (2p at partitions 0:64, 2p+1 at 64:128) ----
            for p in range(NT):
                h0, h1 = 2 * p, 2 * p + 1
                att0 = psum.tile([P, 512], f32, tag="ps", name=f"att0_{p}")
                att1 = psum.tile([P, 512], f32, tag="ps", name=f"att1_{p}")
                for s in range(NT):
                    sc0 = psum.tile([P, 512], f32, tag="ps", name=f"sc0_{p}_{s}")
                    sc1 = psum.tile([P, 512], f32, tag="ps", name=f"sc1_{p}_{s}")
                    nc.tensor.matmul(
                        sc0[:],
                        lhsT=KT_sb[0:64, p, P * s : P * (s + 1)],
                        rhs=QT_sb[0:64, p, :],
                        start=True,
                        stop=True,
                    )
                    nc.tensor.matmul(
                        sc1[:],
                        lhsT=KT_sb[64:128, p, P * s : P * (s + 1)],
                        rhs=QT_sb[64:128, p, :],
                        start=True,
                        stop=True,
                    )
                    e0 = expp.tile([P, 512], f32r, tag="exp", name=f"e0_{p}_{s}")
                    e1 = expp.tile([P, 512], f32r, tag="exp", name=f"e1_{p}_{s}")
                    nc.scalar.activation(e0[:], sc0[:], AF.Exp, scale=0.125)
                    nc.scalar.activation(e1[:], sc1[:], AF.Exp, scale=0.125)
                    nc.tensor.matmul(
                        att0[0 : DH + 1, :],
                        lhsT=Vaug_sb[:, s, 65 * h0 : 65 * h0 + 65],
                        rhs=e0[:],
                        start=(s == 0),
                        stop=(s == NT - 1),
                    )
                    nc.tensor.matmul(
                        att1[0 : DH + 1, :],
                        lhsT=Vaug_sb[:, s, 65 * h1 : 65 * h1 + 65],
                        rhs=e1[:],
                        start=(s == 0),
                        stop=(s == NT - 1),
                    )
                # normalize: attn = att[0:64] * (1 / att[64]) broadcast over partitions
                r0 = small.tile([1, 512], f32, tag="recip")
                nc.vector.reciprocal(r0[:], att0[DH : DH + 1, :])
                b0 = small.tile([64, 512], f32, tag="bc")
                nc.gpsimd.partition_broadcast(b0[:], r0[:])
                nc.vector.tensor_mul(attn_sb[0:64, p, :], att0[0:DH, :], b0[:])

                r1 = small.tile([1, 512], f32, tag="recip")
                nc.vector.reciprocal(r1[:], att1[DH : DH + 1, :])
                b1 = small.tile([64, 512], f32, tag="bc")
                nc.gpsimd.partition_broadcast(b1[:], r1[:])
                t1 = small.tile([64, 512], f32r, tag="tmp")
                nc.vector.tensor_mul(t1[:], att1[0:DH, :], b1[:])
                nc.sync.dma_start(attn_sb[64:128, p, :], t1[:])

            # ---- output projection: outT[128m+i, t] = sum_d Wo[d, 128m+i] attn[d, t] ----
            ops = {}
            for kt in range(NT):
                w_s = stream.tile([P, D], f32r, tag="wstrip")
                nc.sync.dma_start(w_s[:], r(Wo_d[P * kt : P * (kt + 1), :]))
                for m in range(NT):
                    if kt == 0:
                        ops[m] = psum.tile([P, 512], f32, tag="ps", name=f"ops{m}")
                    nc.tensor.matmul(
                        ops[m][:],
                        lhsT=w_s[:, P * m : P * (m + 1)],
                        rhs=attn_sb[:, kt, :],
                        start=(kt == 0),
                        stop=(kt == NT - 1),
                    )
            for m in range(NT):
                ot = small.tile([P, 512], f32, tag="osb")
                nc.scalar.activation(ot[:], ops[m][:], AF.Identity, bias=bo_sb[:, m, :])
                nc.sync.dma_start(outT_d[P * m : P * (m + 1), :], ot[:])

    nc.compile()
    return nc


def _get_nc():
    if "nc" not in _CACHE:
        _CACHE["nc"] = _build()
    return _CACHE["nc"]


def kernel(q_in, k_in, v_in, Wq, bq, Wk, bk, Wv, bv, Wo, bo):
    from concourse.bass_utils import run_bass_kernel_spmd

    q_in = np.asarray(q_in, dtype=np.float32)
    k_in = np.asarray(k_in, dtype=np.float32)
    v_in = np.asarray(v_in, dtype=np.float32)
    Wq = np.ascontiguousarray(np.asarray(Wq, dtype=np.float32))
    Wk = np.ascontiguousarray(np.asarray(Wk, dtype=np.float32))
    Wv = np.ascontiguousarray(np.asarray(Wv, dtype=np.float32))
    Wo = np.ascontiguousarray(np.asarray(Wo, dtype=np.float32))
    bq = np.asarray(bq, dtype=np.float32)
    bk = np.asarray(bk, dtype=np.float32)
    bv = np.asarray(bv, dtype=np.float32)
    bo = np.asarray(bo, dtype=np.float32)

    B = q_in.shape[0]
    # softmax rows sum to 1, so V's bias passes through attention unchanged
    # and folds into the output projection's bias.
    bo_eff = (bv @ Wo + bo).astype(np.float32).reshape(D, 1)
    bq_c = bq.reshape(D, 1)
    bk_c = bk.reshape(D, 1)

    nc = _get_nc()

    in_maps = []
    for i in range(N_CORES):
        b, half = i // 2, i % 2
        rows = slice(SQ * half, SQ * (half + 1))
        in_maps.append(
            {
                "qT": np.ascontiguousarray(q_in[b, rows, :].T),
                "kT": np.ascontiguousarray(k_in[b].T),
                "vT": np.ascontiguousarray(v_in[b].T),
                "Wq": Wq,
                "Wk": Wk,
                "Wv": Wv,
                "Wo": Wo,
                "bq": bq_c,
                "bk": bk_c,
                "bo_eff": bo_eff,
                "ones": np.ones((P, NT, H), dtype=np.float32),
            }
        )

    res = run_bass_kernel_spmd(
        nc, in_maps, core_ids=list(range(N_CORES)), trace=TRACE, tmpdir=TMPDIR
    )
    _CACHE["last"] = res

    out = np.empty((B, S, D), dtype=np.float32)
    for i in range(N_CORES):
        b, half = i // 2, i % 2
        out[b, SQ * half : SQ * (half + 1), :] = res.results[i]["outT"].T
    return out


# revision 17
# speedup vs baseline: 1.4930x; 1.4930x over previous
"""MultiHeadAttention (B=4, S=1024, D=1024, H=16) on 8 TRN2 NeuronCores.

Sharding (no collectives): core i handles batch b=i//2 and query-row half
i%2 (512 query tokens). K/V projections for the batch are duplicated across
the two cores sharing it; each core computes its 512 output rows fully.

Layouts on device (all "transposed"/feature-major so the d_model contraction
sits on SBUF partitions):
  qT  [1024, 512]   q_in[b, rows].T          (host-transposed)
  kT  [1024, 1024]  k_in[b].T
  vT  [1024, 1024]  v_in[b].T
  W*  [1024, 1024]  natural [d_in, d_out]
  outT[1024, 512]   -> host transposes back

Per-head attention with scores kept transposed (Sk on partitions, Sq free)
so softmax(P)@V needs no on-chip transposes; the softmax denominator comes
for free from a ones-column appended to each head's V block; normalization
is folded in post-V (1/denom commutes with the V matmul per query column).
All matmuls run as float32r (1 cycle/row at N=512, ~fp22 operand precision).

All PSUM tiles are [128, 1024] two-bank pairs (4 bufs = all 8 banks); exps
run one [128, 1024] activation per head pair; attention PSUM is drained to
SBUF by a short DVE copy so the banks free before the normalization chain.
"""

import sys

sys.path.insert(0, "/opt/trn_rl_repo")

import numpy as np

D = 1024
H = 16
DH = 64
P = 128
SQ = 512      # query tokens per core
S = 1024      # kv tokens per core (full batch)
NT = 8        # number of 128-wide tiles along d_model
N_CORES = 8

_CACHE = {}
TRACE = False  # set True (e.g. from test.py) to capture an NTFF profile
TMPDIR = None  # where to keep NEFF/NTFF artifacts when tracing


def _build():
    import concourse.bacc as bacc
    import concourse.mybir as mybir
    import concourse.tile as tile

    f32 = mybir.dt.float32
    f32r = mybir.dt.float32r
    AF = mybir.ActivationFunctionType

    nc = bacc.Bacc("TRN2", target_bir_lowering=False, debug=False, num_devices=N_CORES)

    qT_d = nc.dram_tensor("qT", [D, SQ], f32, kind="ExternalInput")
    kT_d = nc.dram_tensor("kT", [D, S], f32, kind="ExternalInput")
    vT_d = nc.dram_tensor("vT", [D, S], f32, kind="ExternalInput")
    Wq_d = nc.dram_tensor("Wq", [D, D], f32, kind="ExternalInput")
    Wk_d = nc.dram_tensor("Wk", [D, D], f32, kind="ExternalInput")
    Wv_d = nc.dram_tensor("Wv", [D, D], f32, kind="ExternalInput")
    Wo_d = nc.dram_tensor("Wo", [D, D], f32, kind="ExternalInput")
    bq_d = nc.dram_tensor("bq", [D, 1], f32, kind="ExternalInput")
    bk_d = nc.dram_tensor("bk", [D, 1], f32, kind="ExternalInput")
    bo_d = nc.dram_tensor("bo_eff", [D, 1], f32, kind="ExternalInput")
    ones_d = nc.dram_tensor("ones", [P, NT, H], f32, kind="ExternalInput")
    outT_d = nc.dram_tensor("outT", [D, SQ], f32, kind="ExternalOutput")

    def r(ap):
        return ap.bitcast(f32r)

    with tile.TileContext(nc) as tc:
        with (
            tc.tile_pool(name="res", bufs=1) as res,
            tc.tile_pool(name="res2", bufs=1) as res2,
            tc.tile_pool(name="stream", bufs=4) as stream,
            tc.tile_pool(name="expp", bufs=4) as expp,
            tc.tile_pool(name="small", bufs=2) as small,
            tc.tile_pool(name="psum", bufs=4, space="PSUM") as psum,
        ):
            # ---- resident SBUF tensors ----
            QT_sb = res.tile([P, NT, SQ], f32r, tag="QT")        # [d_part, m, sq]
            KT_sb = res.tile([P, NT, S], f32r, tag="KT")         # [d_part, m, sk]
            Vaug_sb = res.tile([P, NT, H * (DH + 1)], f32r, tag="Vaug")  # [sk_part, m, 16*65]
            attn_sb = res.tile([P, NT, SQ], f32r, tag="attn")    # [d_part, kt, sq]
            bq_sb = res.tile([P, NT, 1], f32, tag="bq")
            bk_sb = res.tile([P, NT, 1], f32, tag="bk")
            bo_sb = res.tile([P, NT, 1], f32, tag="bo")
            ones_sb = res.tile([P, NT, H], f32, tag="ones")

            nc.sync.dma_start(bq_sb[:], bq_d[:].rearrange("(t p) c -> p t c", p=P))
            nc.sync.dma_start(bk_sb[:], bk_d[:].rearrange("(t p) c -> p t c", p=P))
            nc.sync.dma_start(bo_sb[:], bo_d[:].rearrange("(t p) c -> p t c", p=P))
            nc.sync.dma_start(ones_sb[:], ones_d[:])

            # ---- Q projection: QT[128m+i, t] = sum_k Wq[k, 128m+i] qT[k, t] ----
            qps = {}
            for k in range(NT):
                w_s = stream.tile([P, D], f32r, tag="wstrip")
                nc.sync.dma_start(w_s[:], r(Wq_d[P * k : P * (k + 1), :]))
                x_s = stream.tile([P, SQ], f32r, tag="xstrip")
                nc.sync.dma_start(x_s[:], r(qT_d[P * k : P * (k + 1), :]))
                for m in range(NT):
                    if k == 0 and m % 2 == 0:
                        qps[m // 2] = psum.tile([P, 1024], f32, tag="ps", name=f"qps{m//2}")
                    nc.tensor.matmul(
                        qps[m // 2][:, 512 * (m % 2) : 512 * (m % 2) + 512],
                        lhsT=w_s[:, P * m : P * (m + 1)],
                        rhs=x_s[:],
                        start=(k == 0),
                        stop=(k == NT - 1),
                    )
            for m in range(NT):
                nc.scalar.activation(
                    QT_sb[:, m, :],
                    qps[m // 2][:, 512 * (m % 2) : 512 * (m % 2) + 512],
                    AF.Identity,
                    bias=bq_sb[:, m, :],
                )

            # ones columns of V_aug via DVE strided copy from the staged tile
            # (direct scatter-DMA costs ~16k 4B descriptors and clogs the queue)
            ones_cols = Vaug_sb[:].rearrange("p m (h c) -> p m h c", c=DH + 1)[
                :, :, :, DH
            ]
            nc.vector.tensor_copy(ones_cols, ones_sb[:])

            # ---- K projection ----
            # kT resident in SBUF; Wk streamed once as half-strips.
            # m-split halves keep PSUM demand at 4 pair-tiles (4m x 2n each).
            kT_res = res2.tile([P, NT, S], f32r, tag="kT")
            for k in range(NT):
                nc.sync.dma_start(kT_res[:, k, :], r(kT_d[P * k : P * (k + 1), :]))
            for half in range(2):
                kps = {}
                for k in range(NT):
                    w_s = stream.tile([P, 512], f32r, tag="xstrip", name=f"wkh{half}_{k}")
                    nc.sync.dma_start(
                        w_s[:], r(Wk_d[P * k : P * (k + 1), 512 * half : 512 * (half + 1)])
                    )
                    for mm in range(4):
                        m = 4 * half + mm
                        if k == 0:
                            kps[m] = psum.tile([P, 1024], f32, tag="ps", name=f"kps{m}")
                        for n in range(2):
                            nc.tensor.matmul(
                                kps[m][:, 512 * n : 512 * (n + 1)],
                                lhsT=w_s[:, P * mm : P * (mm + 1)],
                                rhs=kT_res[:, k, 512 * n : 512 * (n + 1)],
                                start=(k == 0),
                                stop=(k == NT - 1),
                            )
                for mm in range(4):
                    m = 4 * half + mm
                    # one copy per m-tile: [128, 1024] (both Sk halves share bias)
                    nc.scalar.activation(
                        KT_sb[:, m, :], kps[m][:], AF.Identity, bias=bk_sb[:, m, :]
                    )

            # ---- V projection, natural layout [token, d], into augmented tiles ----
            # V[128m+i, d] = sum_k vT[k, 128m+i] Wv[k, d]; n-split over d halves
            # (vT full strips streamed per half, Wv half-strips once).
            for n in range(2):
                vps = {}
                for k in range(NT):
                    v_s = stream.tile([P, S], f32r, tag="wstrip", name=f"vs{n}_{k}")
                    nc.sync.dma_start(v_s[:], r(vT_d[P * k : P * (k + 1), :]))
                    w_s = stream.tile([P, 512], f32r, tag="xstrip", name=f"wvh{n}_{k}")
                    nc.sync.dma_start(
                        w_s[:], r(Wv_d[P * k : P * (k + 1), 512 * n : 512 * (n + 1)])
                    )
                    for m in range(NT):
                        if k == 0 and m % 2 == 0:
                            vps[m // 2] = psum.tile(
                                [P, 1024], f32, tag="ps", name=f"vps{n}_{m//2}"
                            )
                        nc.tensor.matmul(
                            vps[m // 2][:, 512 * (m % 2) : 512 * (m % 2) + 512],
                            lhsT=v_s[:, P * m : P * (m + 1)],
                            rhs=w_s[:],
                            start=(k == 0),
                            stop=(k == NT - 1),
                        )
                for m in range(NT):
                    dst = Vaug_sb[:, m, 520 * n : 520 * n + 520].rearrange(
                        "p (h c) -> p h c", c=DH + 1
                    )[:, :, 0:DH]
                    src = vps[m // 2][
                        :, 512 * (m % 2) : 512 * (m % 2) + 512
                    ].rearrange("p (h c) -> p h c", c=DH)
                    nc.vector.tensor_copy(dst, src)

            # ---- attention, head pairs (2p at partitions 0:64, 2p+1 at 64:128) ----
            for p in range(NT):
                h0, h1 = 2 * p, 2 * p + 1
                att = psum.tile([P, 1024], f32, tag="ps", name=f"att{p}")
                for s in range(NT):
                    sc = psum.tile([P, 1024], f32, tag="ps", name=f"sc{p}_{s}")
                    nc.tensor.matmul(
                        sc[:, 0:512],
                        lhsT=KT_sb[0:64, p, P * s : P * (s + 1)],
                        rhs=QT_sb[0:64, p, :],
                        start=True,
                        stop=True,
                    )
                    nc.tensor.matmul(
                        sc[:, 512:1024],
                        lhsT=KT_sb[64:128, p, P * s : P * (s + 1)],
                        rhs=QT_sb[64:128, p, :],
                        start=True,
                        stop=True,
                    )
                    e = expp.tile([P, 1024], f32r, tag="exp", name=f"e{p}_{s}")
                    nc.scalar.activation(e[:], sc[:], AF.Exp, scale=0.125)
                    nc.tensor.matmul(
                        att[0 : DH + 1, 0:512],
                        lhsT=Vaug_sb[:, s, 65 * h0 : 65 * h0 + 65],
                        rhs=e[:, 0:512],
                        start=(s == 0),
                        stop=(s == NT - 1),
                    )
                    nc.tensor.matmul(
                        att[0 : DH + 1, 512:1024],
                        lhsT=Vaug_sb[:, s, 65 * h1 : 65 * h1 + 65],
                        rhs=e[:, 512:1024],
                        start=(s == 0),
                        stop=(s == NT - 1),
                    )
                # drain PSUM fast (frees both banks), then normalize from SBUF
                acp = small.tile([DH + 1, 1024], f32, tag="attcp", name=f"acp{p}")
                nc.vector.tensor_copy(acp[:], att[0 : DH + 1, :])
                rc = small.tile([1, 1024], f32, tag="recip", name=f"rc{p}")
                nc.vector.reciprocal(rc[:], acp[DH : DH + 1, :])
                bc = small.tile([64, 1024], f32, tag="bc", name=f"bc{p}")
                nc.gpsimd.partition_broadcast(bc[:], rc[:])
                nc.vector.tensor_mul(
                    attn_sb[0:64, p, :], acp[0:DH, 0:512], bc[:, 0:512]
                )
                t1 = small.tile([64, 512], f32r, tag="tmp", name=f"t1{p}")
                nc.vector.tensor_mul(t1[:], acp[0:DH, 512:1024], bc[:, 512:1024])
                nc.sync.dma_start(attn_sb[64:128, p, :], t1[:])

            # ---- output projection: outT[128m+i, t] = sum_d Wo[d, 128m+i] attn[d, t] ----
            ops = {}
            for kt in range(NT):
                w_s = stream.tile([P, D], f32r, tag="wstrip")
                nc.sync.dma_start(w_s[:], r(Wo_d[P * kt : P * (kt + 1), :]))
                for m in range(NT):
                    if kt == 0 and m % 2 == 0:
                        ops[m // 2] = psum.tile([P, 1024], f32, tag="ps", name=f"ops{m//2}")
                    nc.tensor.matmul(
                        ops[m // 2][:, 512 * (m % 2) : 512 * (m % 2) + 512],
                        lhsT=w_s[:, P * m : P * (m + 1)],
                        rhs=attn_sb[:, kt, :],
                        start=(kt == 0),
                        stop=(kt == NT - 1),
                    )
            for m in range(NT):
                ot = stream.tile([P, 512], f32, tag="xstrip", name=f"ot{m}")
                nc.scalar.activation(
                    ot[:],
                    ops[m // 2][:, 512 * (m % 2) : 512 * (m % 2) + 512],
                    AF.Identity,
                    bias=bo_sb[:, m, :],
                )
                nc.sync.dma_start(outT_d[P * m : P * (m + 1), :], ot[:])

    nc.compile()
    return nc


def _get_nc():
    if "nc" not in _CACHE:
        _CACHE["nc"] = _build()
    return _CACHE["nc"]


def kernel(q_in, k_in, v_in, Wq, bq, Wk, bk, Wv, bv, Wo, bo):
    from concourse.bass_utils import run_bass_kernel_spmd

    q_in = np.asarray(q_in, dtype=np.float32)
    k_in = np.asarray(k_in, dtype=np.float32)
    v_in = np.asarray(v_in, dtype=np.float32)
    Wq = np.ascontiguousarray(np.asarray(Wq, dtype=np.float32))
    Wk = np.ascontiguousarray(np.asarray(Wk, dtype=np.float32))
    Wv = np.ascontiguousarray(np.asarray(Wv, dtype=np.float32))
    Wo = np.ascontiguousarray(np.asarray(Wo, dtype=np.float32))
    bq = np.asarray(bq, dtype=np.float32)
    bk = np.asarray(bk, dtype=np.float32)
    bv = np.asarray(bv, dtype=np.float32)
    bo = np.asarray(bo, dtype=np.float32)

    B = q_in.shape[0]
    # softmax rows sum to 1, so V's bias passes through attention unchanged
    # and folds into the output projection's bias.
    bo_eff = (bv @ Wo + bo).astype(np.float32).reshape(D, 1)
    bq_c = bq.reshape(D, 1)
    bk_c = bk.reshape(D, 1)

    nc = _get_nc()

    in_maps = []
    for i in range(N_CORES):
        b, half = i // 2, i % 2
        rows = slice(SQ * half, SQ * (half + 1))
        in_maps.append(
            {
                "qT": np.ascontiguousarray(q_in[b, rows, :].T),
                "kT": np.ascontiguousarray(k_in[b].T),
                "vT": np.ascontiguousarray(v_in[b].T),
                "Wq": Wq,
                "Wk": Wk,
                "Wv": Wv,
                "Wo": Wo,
                "bq": bq_c,
                "bk": bk_c,
                "bo_eff": bo_eff,
                "ones": np.ones((P, NT, H), dtype=np.float32),
            }
        )

    res = run_bass_kernel_spmd(
        nc, in_maps, core_ids=list(range(N_CORES)), trace=TRACE, tmpdir=TMPDIR
    )
    _CACHE["last"] = res

    out = np.empty((B, S, D), dtype=np.float32)
    for i in range(N_CORES):
        b, half = i // 2, i % 2
        out[b, SQ * half : SQ * (half + 1), :] = res.results[i]["outT"].T
    return out


# revision 19
# speedup vs baseline: 1.7772x; 1.1903x over previous
"""MultiHeadAttention (B=4, S=1024, D=1024, H=16) on 8 TRN2 NeuronCores.

Sharding (no collectives): core i handles batch b=i//2 and query-row half
i%2 (512 query tokens). K/V projections for the batch are duplicated across
the two cores sharing it; each core computes its 512 output rows fully.

Layouts on device (all "transposed"/feature-major so the d_model contraction
sits on SBUF partitions):
  qT  [1024, 512]   q_in[b, rows].T  (host-transposed, bf16)
  kT  [1024, 1024]  k_in[b].T        (bf16)
  vT  [1024, 1024]  v_in[b].T        (bf16)
  W*  [1024, 1024]  natural [d_in, d_out] (bf16)
  outT[1024, 512]   fp32 -> host transposes back

Per-head attention with scores kept transposed (Sk on partitions, Sq free)
so softmax(P)@V needs no on-chip transposes; the softmax denominator comes
for free from a ones-column appended to each head's V block; normalization
is folded in post-V (1/denom commutes with the V matmul per query column).

Matmul operands are bf16 (fp32 weights would serialize a two-pass
LDWEIGHTS against every matmul: measured 396ns/MM vs 213ns streaming);
accumulation and the softmax denominator chain stay fp32 in PSUM.
All PSUM tiles are [128, 1024] two-bank pairs (4 bufs = all 8 banks); exps
run one [128, 1024] activation per head pair; attention PSUM is drained to
SBUF by a short DVE copy so the banks free before the normalization chain.
"""

import sys

sys.path.insert(0, "/opt/trn_rl_repo")

import numpy as np

D = 1024
H = 16
DH = 64
P = 128
SQ = 512      # query tokens per core
S = 1024      # kv tokens per core (full batch)
NT = 8        # number of 128-wide tiles along d_model
N_CORES = 8

_CACHE = {}
TRACE = False  # set True (e.g. from test.py) to capture an NTFF profile
TMPDIR = None  # where to keep NEFF/NTFF artifacts when tracing


def _build():
    import concourse.bacc as bacc
    import concourse.mybir as mybir
    import concourse.tile as tile

    f32 = mybir.dt.float32
    bf16 = mybir.dt.bfloat16
    AF = mybir.ActivationFunctionType

    nc = bacc.Bacc("TRN2", target_bir_lowering=False, debug=False, num_devices=N_CORES)

    qT_d = nc.dram_tensor("qT", [D, SQ], bf16, kind="ExternalInput")
    kT_d = nc.dram_tensor("kT", [D, S], bf16, kind="ExternalInput")
    vT_d = nc.dram_tensor("vT", [D, S], bf16, kind="ExternalInput")
    Wq_d = nc.dram_tensor("Wq", [D, D], bf16, kind="ExternalInput")
    Wk_d = nc.dram_tensor("Wk", [D, D], bf16, kind="ExternalInput")
    Wv_d = nc.dram_tensor("Wv", [D, D], bf16, kind="ExternalInput")
    Wo_d = nc.dram_tensor("Wo", [D, D], bf16, kind="ExternalInput")
    bq_d = nc.dram_tensor("bq", [P, NT], f32, kind="ExternalInput")
    bk_d = nc.dram_tensor("bk", [P, NT], f32, kind="ExternalInput")
    bo_d = nc.dram_tensor("bo_eff", [P, NT], f32, kind="ExternalInput")
    ones_d = nc.dram_tensor("ones", [P, NT, H], bf16, kind="ExternalInput")
    outT_d = nc.dram_tensor("outT", [D, SQ], f32, kind="ExternalOutput")

    with tile.TileContext(nc) as tc:
        with (
            tc.tile_pool(name="res", bufs=1) as res,
            tc.tile_pool(name="res2", bufs=1) as res2,
            tc.tile_pool(name="stream", bufs=6) as stream,
            tc.tile_pool(name="expp", bufs=6) as expp,
            tc.tile_pool(name="small", bufs=2) as small,
            tc.tile_pool(name="psum", bufs=4, space="PSUM") as psum,
        ):
            # ---- resident SBUF tensors ----
            QT_sb = res.tile([P, NT, SQ], bf16, tag="QT")        # [d_part, m, sq]
            KT_sb = res.tile([P, NT, S], bf16, tag="KT")         # [d_part, m, sk]
            Vaug_sb = res.tile([P, NT, H * (DH + 1)], bf16, tag="Vaug")  # [sk_part, m, 16*65]
            attn_sb = res.tile([P, NT, SQ], bf16, tag="attn")    # [d_part, kt, sq]
            bq_sb = res.tile([P, NT], f32, tag="bq")
            bk_sb = res.tile([P, NT], f32, tag="bk")
            bo_sb = res.tile([P, NT], f32, tag="bo")
            ones_sb = res.tile([P, NT, H], bf16, tag="ones")

            # ---- Q projection: QT[128m+i, t] = sum_k Wq[k, 128m+i] qT[k, t] ----
            qps = {}
            for k in range(NT):
                w_s = stream.tile([P, D], bf16, tag="wstrip")
                nc.sync.dma_start(w_s[:], Wq_d[P * k : P * (k + 1), :])
                x_s = stream.tile([P, SQ], bf16, tag="xstrip")
                nc.sync.dma_start(x_s[:], qT_d[P * k : P * (k + 1), :])
                for m in range(NT):
                    if k == 0 and m % 2 == 0:
                        qps[m // 2] = psum.tile([P, 1024], f32, tag="ps", name=f"qps{m//2}")
                    nc.tensor.matmul(
                        qps[m // 2][:, 512 * (m % 2) : 512 * (m % 2) + 512],
                        lhsT=w_s[:, P * m : P * (m + 1)],
                        rhs=x_s[:],
                        start=(k == 0),
                        stop=(k == NT - 1),
                    )
            # small/scattered loads, emitted late so they don't head-block the
            # DMA queues ahead of the first weight strips
            nc.sync.dma_start(bq_sb[:], bq_d[:])
            nc.sync.dma_start(bk_sb[:], bk_d[:])
            nc.sync.dma_start(bo_sb[:], bo_d[:])
            nc.sync.dma_start(ones_sb[:], ones_d[:])
            for m in range(NT):
                nc.scalar.activation(
                    QT_sb[:, m, :],
                    qps[m // 2][:, 512 * (m % 2) : 512 * (m % 2) + 512],
                    AF.Identity,
                    bias=bq_sb[:, m : m + 1],
                )

            # ones columns of V_aug via DVE strided copy from the staged tile
            # (direct scatter-DMA costs ~16k 4B descriptors and clogs the queue)
            ones_cols = Vaug_sb[:].rearrange("p m (h c) -> p m h c", c=DH + 1)[
                :, :, :, DH
            ]
            nc.vector.tensor_copy(ones_cols, ones_sb[:])

            # ---- K projection ----
            # kT resident in SBUF; Wk streamed once as half-strips; one
            # N=1024 matmul per (k, m) into a two-bank pair tile.
            kT_res = res2.tile([P, NT, S], bf16, tag="kT")
            for k in range(NT):
                nc.sync.dma_start(kT_res[:, k, :], kT_d[P * k : P * (k + 1), :])
            for half in range(2):
                kps = {}
                for k in range(NT):
                    w_s = stream.tile([P, 512], bf16, tag="xstrip", name=f"wkh{half}_{k}")
                    nc.sync.dma_start(
                        w_s[:], Wk_d[P * k : P * (k + 1), 512 * half : 512 * (half + 1)]
                    )
                    for mm in range(4):
                        m = 4 * half + mm
                        if k == 0:
                            kps[m] = psum.tile([P, 1024], f32, tag="ps", name=f"kps{m}")
                        for n in range(2):
                            nc.tensor.matmul(
                                kps[m][:, 512 * n : 512 * (n + 1)],
                                lhsT=w_s[:, P * mm : P * (mm + 1)],
                                rhs=kT_res[:, k, 512 * n : 512 * (n + 1)],
                                start=(k == 0),
                                stop=(k == NT - 1),
                            )
                for mm in range(4):
                    m = 4 * half + mm
                    nc.scalar.activation(
                        KT_sb[:, m, :], kps[m][:], AF.Identity, bias=bk_sb[:, m : m + 1]
                    )

            # ---- V projection (d-halves) + attention, interleaved so the
            # first four head pairs (d-half 0) start while V half 1 projects.
            def v_proj_half(n):
                vps = {}
                for k in range(NT):
                    v_s = stream.tile([P, S], bf16, tag="wstrip", name=f"vs{n}_{k}")
                    nc.sync.dma_start(v_s[:], vT_d[P * k : P * (k + 1), :])
                    w_s = stream.tile([P, 512], bf16, tag="xstrip", name=f"wvh{n}_{k}")
                    nc.sync.dma_start(
                        w_s[:], Wv_d[P * k : P * (k + 1), 512 * n : 512 * (n + 1)]
                    )
                    for m in range(NT):
                        if k == 0 and m % 2 == 0:
                            vps[m // 2] = psum.tile(
                                [P, 1024], f32, tag="ps", name=f"vps{n}_{m//2}"
                            )
                        nc.tensor.matmul(
                            vps[m // 2][:, 512 * (m % 2) : 512 * (m % 2) + 512],
                            lhsT=v_s[:, P * m : P * (m + 1)],
                            rhs=w_s[:],
                            start=(k == 0),
                            stop=(k == NT - 1),
                        )
                for m in range(NT):
                    dst = Vaug_sb[:, m, 520 * n : 520 * n + 520].rearrange(
                        "p (h c) -> p h c", c=DH + 1
                    )[:, :, 0:DH]
                    src = vps[m // 2][
                        :, 512 * (m % 2) : 512 * (m % 2) + 512
                    ].rearrange("p (h c) -> p h c", c=DH)
                    nc.vector.tensor_copy(dst, src)

            def attention_pair(p):
                h0, h1 = 2 * p, 2 * p + 1
                att = psum.tile([P, 1024], f32, tag="ps", name=f"att{p}")
                for s in range(NT):
                    sc = psum.tile([P, 1024], f32, tag="ps", name=f"sc{p}_{s}")
                    nc.tensor.matmul(
                        sc[:, 0:512],
                        lhsT=KT_sb[0:64, p, P * s : P * (s + 1)],
                        rhs=QT_sb[0:64, p, :],
                        start=True,
                        stop=True,
                    )
                    nc.tensor.matmul(
                        sc[:, 512:1024],
                        lhsT=KT_sb[64:128, p, P * s : P * (s + 1)],
                        rhs=QT_sb[64:128, p, :],
                        start=True,
                        stop=True,
                    )
                    e = expp.tile([P, 1024], bf16, tag="exp", name=f"e{p}_{s}")
                    nc.scalar.activation(e[:], sc[:], AF.Exp, scale=0.125)
                    nc.tensor.matmul(
                        att[0 : DH + 1, 0:512],
                        lhsT=Vaug_sb[:, s, 65 * h0 : 65 * h0 + 65],
                        rhs=e[:, 0:512],
                        start=(s == 0),
                        stop=(s == NT - 1),
                    )
                    nc.tensor.matmul(
                        att[0 : DH + 1, 512:1024],
                        lhsT=Vaug_sb[:, s, 65 * h1 : 65 * h1 + 65],
                        rhs=e[:, 512:1024],
                        start=(s == 0),
                        stop=(s == NT - 1),
                    )
                # drain PSUM fast (frees both banks), then normalize from SBUF
                acp = small.tile([DH + 1, 1024], f32, tag="attcp", name=f"acp{p}")
                nc.vector.tensor_copy(acp[:], att[0 : DH + 1, :])
                rc = small.tile([1, 1024], f32, tag="recip", name=f"rc{p}")
                nc.vector.reciprocal(rc[:], acp[DH : DH + 1, :])
                bc = small.tile([64, 1024], f32, tag="bc", name=f"bc{p}")
                nc.gpsimd.partition_broadcast(bc[:], rc[:])
                nc.vector.tensor_mul(
                    attn_sb[0:64, p, :], acp[0:DH, 0:512], bc[:, 0:512]
                )
                t1 = small.tile([64, 512], bf16, tag="tmp", name=f"t1{p}")
                nc.vector.tensor_mul(t1[:], acp[0:DH, 512:1024], bc[:, 512:1024])
                nc.sync.dma_start(attn_sb[64:128, p, :], t1[:])

            v_proj_half(0)
            for p in range(4):
                attention_pair(p)
            v_proj_half(1)
            for p in range(4, NT):
                attention_pair(p)

            # ---- output projection: outT[128m+i, t] = sum_d Wo[d, 128m+i] attn[d, t] ----
            ops = {}
            for kt in range(NT):
                w_s = stream.tile([P, D], bf16, tag="wstrip")
                nc.sync.dma_start(w_s[:], Wo_d[P * kt : P * (kt + 1), :])
                for m in range(NT):
                    if kt == 0 and m % 2 == 0:
                        ops[m // 2] = psum.tile([P, 1024], f32, tag="ps", name=f"ops{m//2}")
                    nc.tensor.matmul(
                        ops[m // 2][:, 512 * (m % 2) : 512 * (m % 2) + 512],
                        lhsT=w_s[:, P * m : P * (m + 1)],
                        rhs=attn_sb[:, kt, :],
                        start=(kt == 0),
                        stop=(kt == NT - 1),
                    )
            for m in range(NT):
                ot = small.tile([P, 512], f32, tag="osb", name=f"ot{m}")
                nc.scalar.activation(
                    ot[:],
                    ops[m // 2][:, 512 * (m % 2) : 512 * (m % 2) + 512],
                    AF.Identity,
                    bias=bo_sb[:, m : m + 1],
                )
                nc.sync.dma_start(outT_d[P * m : P * (m + 1), :], ot[:])

    nc.compile()
    return nc


def _get_nc():
    if "nc" not in _CACHE:
        _CACHE["nc"] = _build()
    return _CACHE["nc"]


def kernel(q_in, k_in, v_in, Wq, bq, Wk, bk, Wv, bv, Wo, bo):
    import ml_dtypes

    from concourse.bass_utils import run_bass_kernel_spmd

    bf = ml_dtypes.bfloat16
    q_in = np.asarray(q_in, dtype=np.float32)
    k_in = np.asarray(k_in, dtype=np.float32)
    v_in = np.asarray(v_in, dtype=np.float32)
    Wq_b = np.ascontiguousarray(np.asarray(Wq, dtype=np.float32).astype(bf))
    Wk_b = np.ascontiguousarray(np.asarray(Wk, dtype=np.float32).astype(bf))
    Wv_b = np.ascontiguousarray(np.asarray(Wv, dtype=np.float32).astype(bf))
    Wo_b = np.ascontiguousarray(np.asarray(Wo, dtype=np.float32).astype(bf))
    Wo = np.asarray(Wo, dtype=np.float32)
    bq = np.asarray(bq, dtype=np.float32)
    bk = np.asarray(bk, dtype=np.float32)
    bv = np.asarray(bv, dtype=np.float32)
    bo = np.asarray(bo, dtype=np.float32)

    B = q_in.shape[0]
    # softmax rows sum to 1, so V's bias passes through attention unchanged
    # and folds into the output projection's bias.
    bo_eff = (bv @ Wo + bo).astype(np.float32)

    def pack_bias(b):
        # [D] -> [P, NT] with element (p, t) = b[128*t + p]
        return np.ascontiguousarray(b.reshape(NT, P).T)

    nc = _get_nc()

    in_maps = []
    for i in range(N_CORES):
        b, half = i // 2, i % 2
        rows = slice(SQ * half, SQ * (half + 1))
        in_maps.append(
            {
                "qT": np.ascontiguousarray(q_in[b, rows, :].T.astype(bf)),
                "kT": np.ascontiguousarray(k_in[b].T.astype(bf)),
                "vT": np.ascontiguousarray(v_in[b].T.astype(bf)),
                "Wq": Wq_b,
                "Wk": Wk_b,
                "Wv": Wv_b,
                "Wo": Wo_b,
                "bq": pack_bias(bq),
                "bk": pack_bias(bk),
                "bo_eff": pack_bias(bo_eff),
                "ones": np.ones((P, NT, H), dtype=bf),
            }
        )

    res = run_bass_kernel_spmd(
        nc, in_maps, core_ids=list(range(N_CORES)), trace=TRACE, tmpdir=TMPDIR
    )
    _CACHE["last"] = res

    out = np.empty((B, S, D), dtype=np.float32)
    for i in range(N_CORES):
        b, half = i // 2, i % 2
        out[b, SQ * half : SQ * (half + 1), :] = res.results[i]["outT"].T
    return out


# revision 21
# speedup vs baseline: 1.8067x; 1.0166x over previous
"""MultiHeadAttention (B=4, S=1024, D=1024, H=16) on 8 TRN2 NeuronCores.

Sharding (no collectives): core i handles batch b=i//2 and query-row half
i%2 (512 query tokens). K/V projections for the batch are duplicated across
the two cores sharing it; each core computes its 512 output rows fully.

Layouts on device (all "transposed"/feature-major so the d_model contraction
sits on SBUF partitions):
  qT  [1024, 512]   q_in[b, rows].T  (host-transposed, bf16)
  kT  [1024, 1024]  k_in[b].T        (f16)
  vT  [1024, 1024]  v_in[b].T        (f16)
  W*  [1024, 1024]  natural [d_in, d_out] (f16)
  outT[1024, 512]   fp32 -> host transposes back

Per-head attention with scores kept transposed (Sk on partitions, Sq free)
so softmax(P)@V needs no on-chip transposes; the softmax denominator comes
for free from a ones-column appended to each head's V block; normalization
is folded in post-V (1/denom commutes with the V matmul per query column).

Matmul operands are bf16 (fp32 weights would serialize a two-pass
LDWEIGHTS against every matmul: measured 396ns/MM vs 213ns streaming);
accumulation and the softmax denominator chain stay fp32 in PSUM.
All PSUM tiles are [128, 1024] two-bank pairs (4 bufs = all 8 banks); exps
run one [128, 1024] activation per head pair; attention PSUM is drained to
SBUF by a short DVE copy so the banks free before the normalization chain.
"""

import sys

sys.path.insert(0, "/opt/trn_rl_repo")

import numpy as np

D = 1024
H = 16
DH = 64
P = 128
SQ = 512      # query tokens per core
S = 1024      # kv tokens per core (full batch)
NT = 8        # number of 128-wide tiles along d_model
N_CORES = 8

_CACHE = {}
TRACE = False  # set True (e.g. from test.py) to capture an NTFF profile
TMPDIR = None  # where to keep NEFF/NTFF artifacts when tracing


def _build():
    import concourse.bacc as bacc
    import concourse.mybir as mybir
    import concourse.tile as tile

    f32 = mybir.dt.float32
    f16 = mybir.dt.float16
    AF = mybir.ActivationFunctionType

    nc = bacc.Bacc("TRN2", target_bir_lowering=False, debug=False, num_devices=N_CORES)

    qT_d = nc.dram_tensor("qT", [D, SQ], f16, kind="ExternalInput")
    kT_d = nc.dram_tensor("kT", [D, S], f16, kind="ExternalInput")
    vT_d = nc.dram_tensor("vT", [D, S], f16, kind="ExternalInput")
    Wq_d = nc.dram_tensor("Wq", [D, D], f16, kind="ExternalInput")
    Wk_d = nc.dram_tensor("Wk", [D, D], f16, kind="ExternalInput")
    Wv_d = nc.dram_tensor("Wv", [D, D], f16, kind="ExternalInput")
    Wo_d = nc.dram_tensor("Wo", [D, D], f16, kind="ExternalInput")
    bq_d = nc.dram_tensor("bq", [P, NT], f32, kind="ExternalInput")
    bk_d = nc.dram_tensor("bk", [P, NT], f32, kind="ExternalInput")
    bo_d = nc.dram_tensor("bo_eff", [P, NT], f32, kind="ExternalInput")
    ones_d = nc.dram_tensor("ones", [P, NT, H], f16, kind="ExternalInput")
    outT_d = nc.dram_tensor("outT", [D, SQ], f32, kind="ExternalOutput")

    with tile.TileContext(nc) as tc:
        with (
            tc.tile_pool(name="res", bufs=1) as res,
            tc.tile_pool(name="res2", bufs=1) as res2,
            tc.tile_pool(name="stream", bufs=6) as stream,
            tc.tile_pool(name="expp", bufs=6) as expp,
            tc.tile_pool(name="small", bufs=2) as small,
            tc.tile_pool(name="psum", bufs=4, space="PSUM") as psum,
        ):
            # ---- resident SBUF tensors ----
            QT_sb = res.tile([P, NT, SQ], f16, tag="QT")        # [d_part, m, sq]
            KT_sb = res.tile([P, NT, S], f16, tag="KT")         # [d_part, m, sk]
            Vaug_sb = res.tile([P, NT, H * (DH + 1)], f16, tag="Vaug")  # [sk_part, m, 16*65]
            attn_sb = res.tile([P, NT, SQ], f16, tag="attn")    # [d_part, kt, sq]
            bq_sb = res.tile([P, NT], f32, tag="bq")
            bk_sb = res.tile([P, NT], f32, tag="bk")
            bo_sb = res.tile([P, NT], f32, tag="bo")
            ones_sb = res.tile([P, NT, H], f16, tag="ones")

            # ---- Q projection: QT[128m+i, t] = sum_k Wq[k, 128m+i] qT[k, t] ----
            qps = {}
            for k in range(NT):
                w_s = stream.tile([P, D], f16, tag="wstrip")
                nc.sync.dma_start(w_s[:], Wq_d[P * k : P * (k + 1), :])
                x_s = stream.tile([P, SQ], f16, tag="xstrip")
                nc.sync.dma_start(x_s[:], qT_d[P * k : P * (k + 1), :])
                for m in range(NT):
                    if k == 0 and m % 2 == 0:
                        qps[m // 2] = psum.tile([P, 1024], f32, tag="ps", name=f"qps{m//2}")
                    nc.tensor.matmul(
                        qps[m // 2][:, 512 * (m % 2) : 512 * (m % 2) + 512],
                        lhsT=w_s[:, P * m : P * (m + 1)],
                        rhs=x_s[:],
                        start=(k == 0),
                        stop=(k == NT - 1),
                    )
            # small/scattered loads, emitted late so they don't head-block the
            # DMA queues ahead of the first weight strips
            nc.sync.dma_start(bq_sb[:], bq_d[:])
            nc.sync.dma_start(bk_sb[:], bk_d[:])
            nc.sync.dma_start(bo_sb[:], bo_d[:])
            nc.sync.dma_start(ones_sb[:], ones_d[:])
            for m in range(NT):
                nc.scalar.activation(
                    QT_sb[:, m, :],
                    qps[m // 2][:, 512 * (m % 2) : 512 * (m % 2) + 512],
                    AF.Identity,
                    bias=bq_sb[:, m : m + 1],
                )

            # ones columns of V_aug via DVE strided copy from the staged tile
            # (direct scatter-DMA costs ~16k 4B descriptors and clogs the queue)
            ones_cols = Vaug_sb[:].rearrange("p m (h c) -> p m h c", c=DH + 1)[
                :, :, :, DH
            ]
            nc.vector.tensor_copy(ones_cols, ones_sb[:])

            # ---- K projection ----
            # kT resident in SBUF; Wk streamed once as half-strips; one
            # N=1024 matmul per (k, m) into a two-bank pair tile.
            kT_res = res2.tile([P, NT, S], f16, tag="kT")
            for half in range(2):
                kps = {}
                for k in range(NT):
                    w_s = stream.tile([P, 512], f16, tag="xstrip", name=f"wkh{half}_{k}")
                    nc.sync.dma_start(
                        w_s[:], Wk_d[P * k : P * (k + 1), 512 * half : 512 * (half + 1)]
                    )
                    if half == 0:
                        nc.sync.dma_start(
                            kT_res[:, k, :], kT_d[P * k : P * (k + 1), :]
                        )
                    for mm in range(4):
                        m = 4 * half + mm
                        if k == 0:
                            kps[m] = psum.tile([P, 1024], f32, tag="ps", name=f"kps{m}")
                        for n in range(2):
                            nc.tensor.matmul(
                                kps[m][:, 512 * n : 512 * (n + 1)],
                                lhsT=w_s[:, P * mm : P * (mm + 1)],
                                rhs=kT_res[:, k, 512 * n : 512 * (n + 1)],
                                start=(k == 0),
                                stop=(k == NT - 1),
                            )
                for mm in range(4):
                    m = 4 * half + mm
                    nc.scalar.activation(
                        KT_sb[:, m, :], kps[m][:], AF.Identity, bias=bk_sb[:, m : m + 1]
                    )

            # ---- V projection (d-halves) + attention, interleaved so the
            # first four head pairs (d-half 0) start while V half 1 projects.
            def v_proj_half(n):
                vps = {}
                for k in range(NT):
                    v_s = stream.tile([P, S], f16, tag="wstrip", name=f"vs{n}_{k}")
                    nc.sync.dma_start(v_s[:], vT_d[P * k : P * (k + 1), :])
                    w_s = stream.tile([P, 512], f16, tag="xstrip", name=f"wvh{n}_{k}")
                    nc.sync.dma_start(
                        w_s[:], Wv_d[P * k : P * (k + 1), 512 * n : 512 * (n + 1)]
                    )
                    for m in range(NT):
                        if k == 0 and m % 2 == 0:
                            vps[m // 2] = psum.tile(
                                [P, 1024], f32, tag="ps", name=f"vps{n}_{m//2}"
                            )
                        nc.tensor.matmul(
                            vps[m // 2][:, 512 * (m % 2) : 512 * (m % 2) + 512],
                            lhsT=v_s[:, P * m : P * (m + 1)],
                            rhs=w_s[:],
                            start=(k == 0),
                            stop=(k == NT - 1),
                        )
                for m in range(NT):
                    dst = Vaug_sb[:, m, 520 * n : 520 * n + 520].rearrange(
                        "p (h c) -> p h c", c=DH + 1
                    )[:, :, 0:DH]
                    src = vps[m // 2][
                        :, 512 * (m % 2) : 512 * (m % 2) + 512
                    ].rearrange("p (h c) -> p h c", c=DH)
                    nc.vector.tensor_copy(dst, src)

            def attention_pair(p):
                h0, h1 = 2 * p, 2 * p + 1
                att = psum.tile([P, 1024], f32, tag="ps", name=f"att{p}")
                for s in range(NT):
                    sc = psum.tile([P, 1024], f32, tag="ps", name=f"sc{p}_{s}")
                    nc.tensor.matmul(
                        sc[:, 0:512],
                        lhsT=KT_sb[0:64, p, P * s : P * (s + 1)],
                        rhs=QT_sb[0:64, p, :],
                        start=True,
                        stop=True,
                    )
                    nc.tensor.matmul(
                        sc[:, 512:1024],
                        lhsT=KT_sb[64:128, p, P * s : P * (s + 1)],
                        rhs=QT_sb[64:128, p, :],
                        start=True,
                        stop=True,
                    )
                    e = expp.tile([P, 1024], f16, tag="exp", name=f"e{p}_{s}")
                    nc.scalar.activation(e[:], sc[:], AF.Exp, scale=0.125)
                    nc.tensor.matmul(
                        att[0 : DH + 1, 0:512],
                        lhsT=Vaug_sb[:, s, 65 * h0 : 65 * h0 + 65],
                        rhs=e[:, 0:512],
                        start=(s == 0),
                        stop=(s == NT - 1),
                    )
                    nc.tensor.matmul(
                        att[0 : DH + 1, 512:1024],
                        lhsT=Vaug_sb[:, s, 65 * h1 : 65 * h1 + 65],
                        rhs=e[:, 512:1024],
                        start=(s == 0),
                        stop=(s == NT - 1),
                    )
                # drain PSUM fast (frees both banks), then normalize from SBUF
                acp = small.tile([DH + 1, 1024], f32, tag="attcp", name=f"acp{p}")
                nc.vector.tensor_copy(acp[:], att[0 : DH + 1, :])
                rc = small.tile([1, 1024], f32, tag="recip", name=f"rc{p}")
                nc.vector.reciprocal(rc[:], acp[DH : DH + 1, :])
                bc = small.tile([64, 1024], f32, tag="bc", name=f"bc{p}")
                nc.gpsimd.partition_broadcast(bc[:], rc[:])
                nc.vector.tensor_mul(
                    attn_sb[0:64, p, :], acp[0:DH, 0:512], bc[:, 0:512]
                )
                t1 = small.tile([64, 512], f16, tag="tmp", name=f"t1{p}")
                nc.vector.tensor_mul(t1[:], acp[0:DH, 512:1024], bc[:, 512:1024])
                nc.sync.dma_start(attn_sb[64:128, p, :], t1[:])

            v_proj_half(0)
            for p in range(4):
                attention_pair(p)
            v_proj_half(1)
            for p in range(4, NT):
                attention_pair(p)

            # ---- output projection: outT[128m+i, t] = sum_d Wo[d, 128m+i] attn[d, t] ----
            ops = {}
            for kt in range(NT):
                w_s = stream.tile([P, D], f16, tag="wstrip")
                nc.sync.dma_start(w_s[:], Wo_d[P * kt : P * (kt + 1), :])
                for m in range(NT):
                    if kt == 0 and m % 2 == 0:
                        ops[m // 2] = psum.tile([P, 1024], f32, tag="ps", name=f"ops{m//2}")
                    nc.tensor.matmul(
                        ops[m // 2][:, 512 * (m % 2) : 512 * (m % 2) + 512],
                        lhsT=w_s[:, P * m : P * (m + 1)],
                        rhs=attn_sb[:, kt, :],
                        start=(kt == 0),
                        stop=(kt == NT - 1),
                    )
            for m in range(NT):
                ot = small.tile([P, 512], f32, tag="osb", name=f"ot{m}")
                nc.scalar.activation(
                    ot[:],
                    ops[m // 2][:, 512 * (m % 2) : 512 * (m % 2) + 512],
                    AF.Identity,
                    bias=bo_sb[:, m : m + 1],
                )
                nc.sync.dma_start(outT_d[P * m : P * (m + 1), :], ot[:])

    nc.compile()
    return nc


def _get_nc():
    if "nc" not in _CACHE:
        _CACHE["nc"] = _build()
    return _CACHE["nc"]


def kernel(q_in, k_in, v_in, Wq, bq, Wk, bk, Wv, bv, Wo, bo):
    from concourse.bass_utils import run_bass_kernel_spmd

    bf = np.float16
    q_in = np.asarray(q_in, dtype=np.float32)
    k_in = np.asarray(k_in, dtype=np.float32)
    v_in = np.asarray(v_in, dtype=np.float32)
    Wq_b = np.ascontiguousarray(np.asarray(Wq, dtype=np.float32).astype(bf))
    Wk_b = np.ascontiguousarray(np.asarray(Wk, dtype=np.float32).astype(bf))
    Wv_b = np.ascontiguousarray(np.asarray(Wv, dtype=np.float32).astype(bf))
    Wo_b = np.ascontiguousarray(np.asarray(Wo, dtype=np.float32).astype(bf))
    Wo = np.asarray(Wo, dtype=np.float32)
    bq = np.asarray(bq, dtype=np.float32)
    bk = np.asarray(bk, dtype=np.float32)
    bv = np.asarray(bv, dtype=np.float32)
    bo = np.asarray(bo, dtype=np.float32)

    B = q_in.shape[0]
    # softmax rows sum to 1, so V's bias passes through attention unchanged
    # and folds into the output projection's bias.
    bo_eff = (bv @ Wo + bo).astype(np.float32)

    def pack_bias(b):
        # [D] -> [P, NT] with element (p, t) = b[128*t + p]
        return np.ascontiguousarray(b.reshape(NT, P).T)

    nc = _get_nc()

    in_maps = []
    for i in range(N_CORES):
        b, half = i // 2, i % 2
        rows = slice(SQ * half, SQ * (half + 1))
        in_maps.append(
            {
                "qT": np.ascontiguousarray(q_in[b, rows, :].T.astype(bf)),
                "kT": np.ascontiguousarray(k_in[b].T.astype(bf)),
                "vT": np.ascontiguousarray(v_in[b].T.astype(bf)),
                "Wq": Wq_b,
                "Wk": Wk_b,
                "Wv": Wv_b,
                "Wo": Wo_b,
                "bq": pack_bias(bq),
                "bk": pack_bias(bk),
                "bo_eff": pack_bias(bo_eff),
                "ones": np.ones((P, NT, H), dtype=bf),
            }
        )

    res = run_bass_kernel_spmd(
        nc, in_maps, core_ids=list(range(N_CORES)), trace=TRACE, tmpdir=TMPDIR
    )
    _CACHE["last"] = res

    out = np.empty((B, S, D), dtype=np.float32)
    for i in range(N_CORES):
        b, half = i // 2, i % 2
        out[b, SQ * half : SQ * (half + 1), :] = res.results[i]["outT"].T
    return out
